# revision 3
# baseline (speedup 1.0000x reference)
"""Trainium2 Bass kernel: sigmoid(rowdot(tanh(x1@W.T+b), tanh(x2@W.T+b))).

Sharding: pure data-parallel over batch across 8 NeuronCores (B=65536
total -> 8192 rows/core, D_IN=1024, D_PROJ=128).

The kernel is DMA-bound on the activation loads, so the host pre-packs
x1/x2 into fp16 (end-to-end max rel err ~5e-3 vs the 2e-2 gate, measured
on the reference distribution) and into the exact PE-ready transposed
tile layout, halving HBM traffic to 32 MiB/core (~86 us at the measured
~394 GB/s per-NC HBM rate) and eliminating every on-device PE transpose:

  xc[t][p, s*4096 + k*BT + b] = xs[t*BT + b, k*128 + p]   (s=0: x1, 1: x2)

so each 512-row batch tile is ONE contiguous 2 MiB DMA whose chunks land
contraction-on-partitions, ready to be the matmul moving operand. The
last batch tile is host-packed as four 128-row k-major quarters loaded
by eight 256 KiB DMAs (x2 quarter 3 last), so the post-last-byte drain
is just 8 small matmuls + tanh/mul/reduce/sigmoid + one 512 B store.

Per 512-row tile: 8 fp16 matmuls (N=512, 1 cyc/row) accumulate
oT=W.T@x1T chunkwise into PSUM; ACT fuses tanh(po+bias) PSUM->SBUF; same
for x2; DVE multiplies; PE reduces partitions via ones[128,128] matmul
(f32r); ACT sigmoid; 2 KiB store from a rotating partition. PE load is
~3.7 us/tile vs ~5.2 us/tile of DMA, so only the partition reduce needs
manual deferral (emitted between the next tile's two matmul groups) to
avoid an in-order PE stall behind the tanh->mul chain. Load issue order:
x tile 0, wt, then the x stream; bias/ones ride the scalar-engine DMA
ring so the sync ring never stalls the activation stream.
"""

import numpy as np

import concourse.bacc as bacc
import concourse.mybir as mybir
import concourse.tile as tile
from concourse.bass_utils import run_bass_kernel_spmd

N_CORES = 8
B_TOTAL = 65536
BSH = B_TOTAL // N_CORES  # 8192 rows per core
D_IN = 1024
D_PROJ = 128
P = 128
BT = 512                 # batch tile (matmul moving dim)
NBT = BSH // BT          # 16 batch tiles per core
KC = D_IN // P           # 8 contraction chunks
FW = KC * BT             # 4096 free-dim elements per packed half-tile
QT = 4                   # last tile split into 4 quarters
QB = BT // QT            # 128 rows per quarter

F32 = mybir.dt.float32
F32R = mybir.dt.float32r
F16 = mybir.dt.float16


def _build_module():
    nc = bacc.Bacc("TRN2", target_bir_lowering=False, debug=False)

    xc = nc.dram_tensor("xc", [NBT, P, 2 * FW], F16, kind="ExternalInput").ap()
    wt = nc.dram_tensor("wt", [P, KC, D_PROJ], F16, kind="ExternalInput").ap()
    bias = nc.dram_tensor("bias", [P, 1], F32, kind="ExternalInput").ap()
    ones = nc.dram_tensor("ones", [P, P], F32R, kind="ExternalInput").ap()
    out = nc.dram_tensor("out", [BSH], F32, kind="ExternalOutput").ap()

    # Work items: 15 full 512-row tiles + 4 quarter tiles (drain
    # shortening): (kind, tile_idx, sub_idx, row0, nrows)
    items = [("full", t, 0, t * BT, BT) for t in range(NBT - 1)]
    items += [("q", NBT - 1, q, (NBT - 1) * BT + q * QB, QB) for q in range(QT)]

    with tile.TileContext(nc) as tc:
        with (
            tc.tile_pool(name="consts", bufs=1) as cpool,
            tc.tile_pool(name="x", bufs=3) as xpool,
            tc.tile_pool(name="acts", bufs=2) as apool,
            tc.tile_pool(name="po", bufs=6, space="PSUM") as opool,
        ):
            wt_sb = cpool.tile([P, KC, D_PROJ], F16, tag="wt")
            bias_sb = cpool.tile([P, 1], F32, tag="bias")
            ones_sb = cpool.tile([P, P], F32R, tag="ones")

            pending = []

            def flush_pending():
                while pending:
                    prod_p, row0_p, nr_p, idx_p = pending.pop(0)
                    psim = opool.tile([P, nr_p], F32, name="psim", tag="po")
                    nc.tensor.matmul(
                        psim,
                        ones_sb,
                        prod_p,
                        start=True,
                        stop=True,
                        skip_group_check=True,
                    )
                    sig = apool.tile([P, nr_p], F32, tag="sig")
                    nc.scalar.activation(
                        sig, psim, mybir.ActivationFunctionType.Sigmoid
                    )
                    row = (idx_p * 4) % P  # rotate partition -> spread DMA engines
                    nc.scalar.dma_start(
                        out=out[row0_p:row0_p + nr_p].rearrange(
                            "(a n) -> a n", a=1
                        ),
                        in_=sig[row:row + 1, :],
                    )

            def rhs(sb, kind, sub, k, tens):
                base = tens * FW
                if kind == "full":
                    return sb[:, base + k * BT:base + (k + 1) * BT]
                q0 = base + sub * (KC * QB)
                return sb[:, q0 + k * QB:q0 + (k + 1) * QB]

            def mm_group(sb, kind, sub, nrows, tens, mid=None):
                po = opool.tile([P, nrows], F32, name=f"po{tens}", tag="po")
                for k in range(KC):
                    nc.tensor.matmul(
                        po,
                        wt_sb[:, k, :],
                        rhs(sb, kind, sub, k, tens),
                        start=(k == 0),
                        stop=(k == KC - 1),
                        skip_group_check=True,
                    )
                    if k == 2 and mid is not None:
                        mid()
                t_sb = apool.tile([P, nrows], F32, tag=f"t{tens}")
                nc.scalar.activation(
                    t_sb, po, mybir.ActivationFunctionType.Tanh, bias=bias_sb
                )
                return t_sb

            loaded = {}  # tile_idx -> sb

            def load(it):
                kind, t, sub, _, _ = it
                if t in loaded:
                    return
                sb = xpool.tile([P, 2 * FW], F16, tag="sb")
                if kind == "full":
                    nc.sync.dma_start(out=sb, in_=xc[t])
                else:
                    qw = KC * QB  # 1024 free elems per quarter
                    for q in range(QT):
                        for s in range(2):
                            o = s * FW + q * qw
                            nc.sync.dma_start(
                                out=sb[:, o:o + qw],
                                in_=xc[t][:, o:o + qw],
                            )
                loaded[t] = sb

            def compute(it):
                kind, t, sub, row0, nrows = it
                sb = loaded[t]
                # pending reduce of the previous item rides between the
                # two matmul groups so PE never waits on tanh->mul.
                t1 = mm_group(sb, kind, sub, nrows, 0, mid=flush_pending)
                t2 = mm_group(sb, kind, sub, nrows, 1)
                prod = apool.tile([P, nrows], F32R, tag="prod")
                nc.vector.tensor_mul(prod, t1, t2)
                pending.append((prod, row0, nrows, t * QT + sub))

            # Issue order: x tile 0 first (it IS the stream bottleneck),
            # then wt (gates the first matmul), then the rest of the
            # stream; bias/ones go on the scalar ring (needed only at
            # the first tanh/reduce, ~14 us in).
            load(items[0])
            nc.sync.dma_start(out=wt_sb, in_=wt)
            nc.scalar.dma_start(out=bias_sb, in_=bias)
            nc.scalar.dma_start(out=ones_sb, in_=ones)
            for j, it in enumerate(items):
                if j > 0:
                    load(it)
                    compute(items[j - 1])
            compute(items[-1])
            flush_pending()

    nc.compile()
    return nc


_NC_CACHE = None


def _get_module():
    global _NC_CACHE
    if _NC_CACHE is None:
        _NC_CACHE = _build_module()
    return _NC_CACHE


def _pack_x(x):
    """[B, D_IN] fp32 -> [N_CORES, NBT, P, FW] fp16 PE-ready tiles.

    Slot t holds tile t's transposed layout [p, k*BT + b]; the last tile
    is packed as four k-major 128-row quarters [p, q*KC*QB + k*QB + b].
    """
    xh = np.asarray(x, dtype=np.float32).astype(np.float16)
    a = xh.reshape(N_CORES, NBT, BT, KC, P).transpose(0, 1, 4, 3, 2)
    f = np.ascontiguousarray(a).reshape(N_CORES, NBT, P, FW)
    last = a[:, NBT - 1].reshape(N_CORES, P, KC, QT, QB)
    f[:, NBT - 1] = last.transpose(0, 1, 3, 2, 4).reshape(N_CORES, P, FW)
    return f


def _pack_inputs(x1, x2, W, b):
    f1 = _pack_x(x1)
    f2 = _pack_x(x2)
    xc_all = np.concatenate([f1, f2], axis=3)  # [c, t, p, 2*FW]
    wt = np.ascontiguousarray(
        np.asarray(W, dtype=np.float32).T.reshape(KC, P, D_PROJ)
        .transpose(1, 0, 2)
    ).astype(np.float16)
    bias = np.ascontiguousarray(np.asarray(b, dtype=np.float32).reshape(P, 1))
    ones = np.ones((P, P), dtype=np.float32)
    return [
        {
            "xc": np.ascontiguousarray(xc_all[i]),
            "wt": wt,
            "bias": bias,
            "ones": ones,
        }
        for i in range(N_CORES)
    ]


def kernel(x1, x2, W, b):
    nc = _get_module()
    in_maps = _pack_inputs(x1, x2, W, b)
    res = run_bass_kernel_spmd(nc, in_maps, core_ids=list(range(N_CORES)))
    return np.concatenate([res.results[i]["out"] for i in range(N_CORES)])


# revision 8
# speedup vs baseline: 1.0234x; 1.0234x over previous
"""Trainium2 Bass kernel: sigmoid(rowdot(tanh(x1@W.T+b), tanh(x2@W.T+b))).

Sharding: pure data-parallel over batch across 8 NeuronCores (B=65536
total -> 8192 rows/core, D_IN=1024, D_PROJ=128).

The kernel is DMA-bound on the activation loads, so the host pre-packs
x1/x2 into fp16 (end-to-end max rel err ~5e-3 vs the 2e-2 gate, measured
on the reference distribution) and into the exact PE-ready transposed
tile layout, halving HBM traffic to 32 MiB/core (~86 us at the measured
~394 GB/s per-NC HBM rate) and eliminating every on-device PE transpose:

  xc[t][p, s*4096 + k*BT + b] = xs[t*BT + b, k*128 + p]   (s=0: x1, 1: x2)

so each 512-row batch tile is ONE contiguous 2 MiB DMA whose chunks land
contraction-on-partitions, ready to be the matmul moving operand. The
last batch tile is host-packed as four 128-row k-major quarters loaded
by eight 256 KiB DMAs (x2 quarter 3 last), so the post-last-byte drain
is just 8 small matmuls + tanh/mul/reduce/sigmoid + one 512 B store.

Per 512-row tile: 8 fp16 matmuls (N=512, 1 cyc/row) accumulate
oT=W.T@x1T chunkwise into PSUM; ACT fuses tanh(po+bias) PSUM->SBUF; same
for x2; DVE multiplies; PE reduces partitions via ones[128,128] matmul
(f32r); ACT sigmoid; 2 KiB store from a rotating partition. PE load is
~3.7 us/tile vs ~5.2 us/tile of DMA, so only the partition reduce needs
manual deferral (emitted between the next tile's two matmul groups) to
avoid an in-order PE stall behind the tanh->mul chain. Load issue order:
x tile 0, wt, then the x stream; bias/ones ride the scalar-engine DMA
ring so the sync ring never stalls the activation stream.
"""

import numpy as np

import concourse.bacc as bacc
import concourse.mybir as mybir
import concourse.tile as tile
from concourse.bass_utils import run_bass_kernel_spmd

N_CORES = 8
B_TOTAL = 65536
BSH = B_TOTAL // N_CORES  # 8192 rows per core
D_IN = 1024
D_PROJ = 128
P = 128
BT = 512                 # batch tile (matmul moving dim)
NBT = BSH // BT          # 16 batch tiles per core
KC = D_IN // P           # 8 contraction chunks
FW = KC * BT             # 4096 free-dim elements per packed half-tile
QT = 4                   # last tile split into 4 quarters
QB = BT // QT            # 128 rows per quarter

F32 = mybir.dt.float32
F32R = mybir.dt.float32r
F16 = mybir.dt.float16


def _build_module():
    nc = bacc.Bacc("TRN2", target_bir_lowering=False, debug=False)

    xc = nc.dram_tensor("xc", [NBT, P, 2 * FW], F16, kind="ExternalInput").ap()
    wt = nc.dram_tensor("wt", [P, KC, D_PROJ], F16, kind="ExternalInput").ap()
    bias = nc.dram_tensor("bias", [P, 1], F32, kind="ExternalInput").ap()
    ones = nc.dram_tensor("ones", [P, P], F32R, kind="ExternalInput").ap()
    out = nc.dram_tensor("out", [BSH], F32, kind="ExternalOutput").ap()

    # Work items: 16 full 512-row tiles. (kind, tile_idx, sub, row0, nrows)
    items = [("full", t, 0, t * BT, BT) for t in range(NBT)]

    with tile.TileContext(nc) as tc:
        with (
            tc.tile_pool(name="consts", bufs=1) as cpool,
            tc.tile_pool(name="x", bufs=3) as xpool,
            tc.tile_pool(name="acts", bufs=2) as apool,
            tc.tile_pool(name="po", bufs=6, space="PSUM") as opool,
        ):
            wt_sb = cpool.tile([P, KC, D_PROJ], F16, tag="wt")
            bias_sb = cpool.tile([P, 1], F32, tag="bias")
            ones_sb = cpool.tile([P, P], F32R, tag="ones")

            pending = []

            def flush_pending():
                while pending:
                    prod_p, row0_p, nr_p, idx_p = pending.pop(0)
                    psim = opool.tile([P, nr_p], F32, name="psim", tag="po")
                    nc.tensor.matmul(
                        psim,
                        ones_sb,
                        prod_p,
                        start=True,
                        stop=True,
                        skip_group_check=True,
                    )
                    sig = apool.tile([P, nr_p], F32, tag="sig")
                    nc.scalar.activation(
                        sig, psim, mybir.ActivationFunctionType.Sigmoid
                    )
                    row = (idx_p * 4) % P  # rotate partition -> spread DMA engines
                    nc.scalar.dma_start(
                        out=out[row0_p:row0_p + nr_p].rearrange(
                            "(a n) -> a n", a=1
                        ),
                        in_=sig[row:row + 1, :],
                    )

            def rhs(sb, kind, sub, k, tens):
                base = tens * FW
                return sb[:, base + k * BT:base + (k + 1) * BT]

            def mm_group(sb, kind, sub, nrows, tens, mid=None):
                po = opool.tile([P, nrows], F32, name=f"po{tens}", tag="po")
                for k in range(KC):
                    nc.tensor.matmul(
                        po,
                        wt_sb[:, k, :],
                        rhs(sb, kind, sub, k, tens),
                        start=(k == 0),
                        stop=(k == KC - 1),
                        skip_group_check=True,
                    )
                    if k == 2 and mid is not None:
                        mid()
                t_sb = apool.tile([P, nrows], F32, tag=f"t{tens}")
                nc.scalar.activation(
                    t_sb, po, mybir.ActivationFunctionType.Tanh, bias=bias_sb
                )
                return t_sb

            loaded = {}  # tile_idx -> sb

            def load(it):
                kind, t, sub, _, _ = it
                if t in loaded:
                    return
                sb = xpool.tile([P, 2 * FW], F16, tag="sb")
                if t < NBT - 1:
                    nc.sync.dma_start(out=sb, in_=xc[t])
                else:
                    # Last tile: x1 whole, then x2 split k0-5 / k6 / k7 so
                    # the post-last-byte chain is a single N=512 matmul +
                    # tanh/mul/reduce/sigmoid + store (~5 us) instead of a
                    # whole 8-matmul group behind the full 2 MiB DMA.
                    nc.sync.dma_start(out=sb[:, :FW], in_=xc[t][:, :FW])
                    cuts = [0, 6 * BT, 7 * BT, 8 * BT]
                    for a, bnd in zip(cuts[:-1], cuts[1:]):
                        nc.sync.dma_start(
                            out=sb[:, FW + a:FW + bnd],
                            in_=xc[t][:, FW + a:FW + bnd],
                        )
                loaded[t] = sb

            def compute(it):
                kind, t, sub, row0, nrows = it
                sb = loaded[t]
                # pending reduce of the previous item rides between the
                # two matmul groups so PE never waits on tanh->mul.
                t1 = mm_group(sb, kind, sub, nrows, 0, mid=flush_pending)
                t2 = mm_group(sb, kind, sub, nrows, 1)
                prod = apool.tile([P, nrows], F32R, tag="prod")
                nc.vector.tensor_mul(prod, t1, t2)
                pending.append((prod, row0, nrows, t * QT + sub))

            # Issue order: x tile 0 first on the sync ring (it IS the
            # stream bottleneck); wt/bias/ones ride the otherwise-idle
            # scalar ring (wt gates the first matmul ~13 us in, bias the
            # first tanh, ones the first reduce).
            load(items[0])
            nc.scalar.dma_start(out=wt_sb, in_=wt)
            nc.scalar.dma_start(out=bias_sb, in_=bias)
            nc.scalar.dma_start(out=ones_sb, in_=ones)
            for j, it in enumerate(items):
                if j > 0:
                    load(it)
                    compute(items[j - 1])
            compute(items[-1])
            flush_pending()

    nc.compile()
    return nc


_NC_CACHE = None


def _get_module():
    global _NC_CACHE
    if _NC_CACHE is None:
        _NC_CACHE = _build_module()
    return _NC_CACHE


def _pack_x(x):
    """[B, D_IN] fp32 -> [N_CORES, NBT, P, FW] fp16 PE-ready tiles.

    Slot t holds tile t's transposed layout [p, k*BT + b].
    """
    xh = np.asarray(x, dtype=np.float32).astype(np.float16)
    a = xh.reshape(N_CORES, NBT, BT, KC, P).transpose(0, 1, 4, 3, 2)
    return np.ascontiguousarray(a).reshape(N_CORES, NBT, P, FW)


def _pack_inputs(x1, x2, W, b):
    f1 = _pack_x(x1)
    f2 = _pack_x(x2)
    xc_all = np.concatenate([f1, f2], axis=3)  # [c, t, p, 2*FW]
    wt = np.ascontiguousarray(
        np.asarray(W, dtype=np.float32).T.reshape(KC, P, D_PROJ)
        .transpose(1, 0, 2)
    ).astype(np.float16)
    bias = np.ascontiguousarray(np.asarray(b, dtype=np.float32).reshape(P, 1))
    ones = np.ones((P, P), dtype=np.float32)
    return [
        {
            "xc": np.ascontiguousarray(xc_all[i]),
            "wt": wt,
            "bias": bias,
            "ones": ones,
        }
        for i in range(N_CORES)
    ]


def kernel(x1, x2, W, b):
    nc = _get_module()
    in_maps = _pack_inputs(x1, x2, W, b)
    res = run_bass_kernel_spmd(nc, in_maps, core_ids=list(range(N_CORES)))
    return np.concatenate([res.results[i]["out"] for i in range(N_CORES)])
